# revision 1
# baseline (speedup 1.0000x reference)
"""Bass/Trainium2 kernel for BilinearlyModulatedAttention.

Sharding: 8 cores = 2 (batch) x 4 (head groups of 4 heads).
Each core computes, for its batch b and heads [4g, 4g+4):
  QT/KT (feature-major, d x T), V (token-major), bilinear gate, causal
  softmax in transposed layout (scores[s, t]), PV with a ones-column
  giving softmax denominators, normalization, and a partial output
  projection Y_partial = O^T.T @ W_out[rows]. Host sums the 4 partials
  per batch and adds b_out.

Key layout/HW notes:
 - scores are computed TRANSPOSED (s on partitions, t on free dim) so the
   softmax denominator sum_s e[s,t] falls out of the PV matmul via an
   appended ones-column in the stationary operand (M=65).
 - no max-subtraction in softmax: scores are ~N(0,0.4), exp is safe.
 - all matmuls use float32r (1 PE cycle/row vs 4 for float32, ~1.6e-4
   matmul accuracy); use_f32r=False falls back to exact float32.
 - a PSUM bank must only ever receive matmuls of ONE contraction
   row-group base (mixing base-0 / base-64 writes corrupts results), so
   base-64 matmuls (odd heads' scores + gates) get a dedicated pool.
 - custom-DVE ops and partition_broadcast require partition-base-0 APs.
 - sigmoid is computed as 0.5*tanh(x/2)+0.5 (tanh shares the ACT table
   set with exp, avoiding ~2.7us table switches).
"""

import sys

if "/opt/trn_rl_repo" not in sys.path:
    sys.path.insert(0, "/opt/trn_rl_repo")

import numpy as np

D_MODEL = 1024
N_HEADS = 16
D_HEAD = 64
B = 2
T_FULL = 2048
N_CORES = 8
H_LOC = N_HEADS // (N_CORES // B)  # 4 heads per core

_LDW_PATCHED = False


def _patch_ldw_opt():
    """Compile walrus with --enable-ldw-opt=true (elides redundant
    LDWEIGHTS reloads). Wraps concourse.bass_utils.run_command."""
    global _LDW_PATCHED
    if _LDW_PATCHED:
        return
    import concourse.bass_utils as BU
    orig = BU.run_command

    def run_patched(argv, **kw):
        argv = [a.replace("--enable-ldw-opt=false", "--enable-ldw-opt=true")
                if isinstance(a, str) else a for a in argv]
        return orig(argv, **kw)

    BU.run_command = run_patched
    _LDW_PATCHED = True


def build_nc(T=T_FULL, D=D_MODEL, h_loc=H_LOC, dh=D_HEAD, W=512,
             use_f32r=True):
    """Build the Bass module for one core's shard. Returns (nc, meta)."""
    import concourse.bass as bass
    import concourse.mybir as mybir
    import concourse.tile as tile
    from concourse import bacc
    from contextlib import ExitStack
    from collections import deque

    f32 = mybir.dt.float32
    fm = mybir.dt.float32r if use_f32r else f32
    AF = mybir.ActivationFunctionType
    ALU = mybir.AluOpType

    KN = D // 128            # k-tiles for the qkv projections
    TT = T // 128            # 128-token tiles
    assert T % W == 0 and W == 512
    NCH = T // W             # chunks
    W128 = W // 128          # s-tiles per chunk (4)
    DHL = h_loc * dh         # local head dim total (256)
    NP = h_loc // 2          # head pairs
    KO = DHL // 128          # out-proj k-tiles (2)
    VGW = dh + 1             # vg last dim: 64 V cols + ones col
    SCALE = 1.0 / float(np.sqrt(dh))

    nc = bacc.Bacc("TRN2", target_bir_lowering=False, debug=False)

    xt_d = nc.dram_tensor("xt", (128, KN, T), fm, kind="ExternalInput")
    wq_d = nc.dram_tensor("wq", (128, KN, DHL), fm, kind="ExternalInput")
    wk_d = nc.dram_tensor("wk", (128, KN, DHL), fm, kind="ExternalInput")
    wv_d = nc.dram_tensor("wv", (128, KN, DHL), fm, kind="ExternalInput")
    wg_d = nc.dram_tensor("wg", (128, DHL), fm, kind="ExternalInput")
    wo_d = nc.dram_tensor("wo", (128, KO, D), fm, kind="ExternalInput")
    mask_d = nc.dram_tensor("mask", (128, 128), fm, kind="ExternalInput")
    ones_d = nc.dram_tensor("ones", (128, TT), fm, kind="ExternalInput")
    y_d = nc.dram_tensor("y", (T, D), f32, kind="ExternalOutput")

    with ExitStack() as ctx:
        tc = ctx.enter_context(tile.TileContext(nc))
        sb_w = ctx.enter_context(tc.tile_pool(name="wts", bufs=1))
        sb_big = ctx.enter_context(tc.tile_pool(name="big", bufs=1))
        sb_e = ctx.enter_context(tc.tile_pool(name="e", bufs=3))
        sb_sig = ctx.enter_context(tc.tile_pool(name="sig", bufs=2))
        sb_nrm = ctx.enter_context(tc.tile_pool(name="nrm", bufs=1))
        sb_y = ctx.enter_context(tc.tile_pool(name="ysb", bufs=2))
        ps_b0 = ctx.enter_context(
            tc.tile_pool(name="psb0", bufs=2, space=bass.MemorySpace.PSUM))
        ps_b64 = ctx.enter_context(
            tc.tile_pool(name="psb64", bufs=2, space=bass.MemorySpace.PSUM))
        ps_u = ctx.enter_context(
            tc.tile_pool(name="psu", bufs=2, space=bass.MemorySpace.PSUM))

        # ---- persistent SBUF tensors ----
        xt = sb_big.tile([128, KN, T], fm, tag="xt")
        wq = sb_w.tile([128, KN, DHL], fm, tag="wq")
        wk = sb_w.tile([128, KN, DHL], fm, tag="wk")
        wv = sb_w.tile([128, KN, DHL], fm, tag="wv")
        wg = sb_w.tile([128, DHL], fm, tag="wg")
        wo = sb_w.tile([128, KO, D], fm, tag="wo")
        msk = sb_w.tile([128, 128], fm, tag="msk")
        qt = [sb_big.tile([128, T], fm, tag=f"qt{p}", name=f"qt{p}")
              for p in range(NP)]
        kt = [sb_big.tile([128, T], fm, tag=f"kt{p}", name=f"kt{p}")
              for p in range(NP)]
        ot = [sb_big.tile([128, T], fm, tag=f"ot{p}", name=f"ot{p}")
              for p in range(NP)]
        vg = sb_big.tile([128, TT, h_loc, VGW], fm, tag="vg")

        # ---- input DMAs: interleave per-k weight planes with xt k-planes
        # so the first matmuls can start within a few microseconds ----
        for k in range(KN):
            nc.sync.dma_start(wq[:, k, :], wq_d[:, k, :])
            nc.sync.dma_start(wk[:, k, :], wk_d[:, k, :])
            nc.sync.dma_start(xt[:, k, :], xt_d[:, k, :])
        nc.sync.dma_start(wv[:], wv_d[:])
        nc.sync.dma_start(wg[:], wg_d[:])
        nc.sync.dma_start(msk[:], mask_d[:])
        for s in range(h_loc):
            nc.sync.dma_start(vg[:, :, s, dh], ones_d[:])
        nc.sync.dma_start(wo[:], wo_d[:])

        # ---- phase-A jobs ----
        def qk_job(w_sb, dst, p, c, nch=1):
            # nch chunks share each k's LDWEIGHTS (consecutive same-lhsT
            # matmuls are elided by --enable-ldw-opt=true)
            pss = [ps_b0.tile([128, W], f32, tag="b0", name=f"qkps{cc}")
                   for cc in range(nch)]
            for k in range(KN):
                for cc in range(nch):
                    nc.tensor.matmul(
                        pss[cc][:], w_sb[:, k, 128 * p:128 * p + 128],
                        xt[:, k, (c + cc) * W:(c + cc + 1) * W],
                        start=(k == 0), stop=(k == KN - 1),
                        skip_group_check=True)
            for cc in range(nch):
                nc.vector.tensor_copy(
                    dst[:, (c + cc) * W:(c + cc + 1) * W], pss[cc][:])

        def vg_job(ti):
            # one base-0 psum tile: V in cols [0:DHL), j=0 gates in
            # [DHL:DHL+128). j=1 gates go to the base-64 pool.
            vps = ps_b0.tile([128, W], f32, tag="b0")
            for k in range(KN):
                nc.tensor.matmul(
                    vps[:, :DHL],
                    xt[:, k, 128 * ti:128 * ti + 128],
                    wv[:, k, :],
                    start=(k == 0), stop=(k == KN - 1),
                    skip_group_check=True)
            for p in range(NP):
                h = 2 * p
                nc.tensor.matmul(
                    vps[:, DHL + 64 * p:DHL + 64 * p + 64],
                    qt[p][0:64, 128 * ti:128 * ti + 128],
                    wg[0:64, dh * h:dh * h + dh],
                    start=True, stop=True, skip_group_check=True)
            gps1 = ps_b64.tile([128, W], f32, tag="b64")
            for p in range(NP):
                h = 2 * p + 1
                nc.tensor.matmul(
                    gps1[:, 64 * p:64 * p + 64],
                    qt[p][64:128, 128 * ti:128 * ti + 128],
                    wg[64:128, dh * h:dh * h + dh],
                    start=True, stop=True, skip_group_check=True)
            # sigmoid(x) = 0.5*tanh(x/2) + 0.5 (stays in the exp table set)
            sig = sb_sig.tile([128, DHL], f32, tag="sig")
            sig4 = sig[:].rearrange("p (a b c) -> p a b c", a=NP, b=2)
            nc.scalar.activation(
                sig4[:, :, 0, :],
                vps[:, DHL:DHL + 128].rearrange("p (a c) -> p a c", a=NP),
                AF.Tanh, scale=0.5)
            nc.scalar.activation(
                sig4[:, :, 1, :],
                gps1[:, 0:128].rearrange("p (a c) -> p a c", a=NP),
                AF.Tanh, scale=0.5)
            nc.vector.tensor_scalar(sig[:], sig[:], 0.5, 0.5,
                                    ALU.mult, ALU.add)
            nc.vector.tensor_mul(
                vg[:, ti, :, 0:dh],
                vps[:, :DHL].rearrange("p (h d) -> p h d", h=h_loc),
                sig[:].rearrange("p (h d) -> p h d", h=h_loc))

        # ---- phase-B inner iteration ----
        def b_iter(c, p, i, UA, UB, S):
            base = c * W128
            off = 128 * (i - base) if i >= base else 0
            sA = ps_b0.tile([128, W], f32, tag="b0")
            sB = ps_b64.tile([128, W], f32, tag="b64")
            for j, sps in ((0, sA), (1, sB)):
                nc.tensor.matmul(
                    sps[:, off:W],
                    kt[p][64 * j:64 * j + 64, 128 * i:128 * i + 128],
                    qt[p][64 * j:64 * j + 64, c * W + off:(c + 1) * W],
                    start=True, stop=True)
            es = []
            for sps in (sA, sB):
                e = sb_e.tile([128, W], fm, tag="e")
                nc.scalar.activation(e[:, off:W], sps[:, off:W], AF.Exp,
                                     scale=SCALE)
                if i >= base:
                    nc.vector.tensor_mul(e[:, off:off + 128],
                                         e[:, off:off + 128], msk[:])
                es.append(e)
            last_i = min(S - 1, base + W128 - 1)
            for j, (e, U) in ((0, (es[0], UA)), (1, (es[1], UB))):
                nc.tensor.matmul(
                    U[0:65, off:W],
                    vg[:, i, 2 * p + j, 0:65],
                    e[:, off:W],
                    start=(i == 0), stop=(i == last_i),
                    skip_group_check=True)

        def normalize(c, p, UA, UB):
            # Denominator rows live at partition 64; custom-DVE ops and
            # partition_broadcast need base-0 APs, so bounce them through
            # a cross-partition SBUF DMA.
            dtA = sb_nrm.tile([65, W], f32, tag="dtA")
            dtB = sb_nrm.tile([65, W], f32, tag="dtB")
            nc.vector.tensor_copy(dtA[64:65, :], UA[64:65, :])
            nc.vector.tensor_copy(dtB[64:65, :], UB[64:65, :])
            den = sb_nrm.tile([2, W], f32, tag="den")
            nc.sync.dma_start(den[0:1, :], dtA[64:65, :])
            nc.sync.dma_start(den[1:2, :], dtB[64:65, :])
            rec = sb_nrm.tile([2, W], f32, tag="rec")
            nc.vector.reciprocal_approx_fast(rec[:], den[:])
            recB = sb_nrm.tile([1, W], f32, tag="recB")
            nc.sync.dma_start(recB[:], rec[1:2, :])
            bcA = sb_nrm.tile([64, W], f32, tag="bcA")
            bcB = sb_nrm.tile([64, W], f32, tag="bcB")
            nc.gpsimd.partition_broadcast(bcA[:], rec[0:1, :])
            nc.gpsimd.partition_broadcast(bcB[:], recB[:])
            nc.vector.tensor_mul(ot[p][0:64, c * W:(c + 1) * W],
                                 UA[0:64, :], bcA[:])
            obB = sb_nrm.tile([64, W], fm, tag="obB")
            nc.vector.tensor_mul(obB[:], UB[0:64, :], bcB[:])
            nc.sync.dma_start(ot[p][64:128, c * W:(c + 1) * W], obB[:])

        # ---- phase-C job (one 128-token tile x one 512-col slab) ----
        def c_job(tt, n):
            yp = ps_b0.tile([128, W], f32, tag="b0")
            for kt_i in range(KO):
                nc.tensor.matmul(
                    yp[:],
                    ot[kt_i][:, 128 * tt:128 * tt + 128],
                    wo[:, kt_i, n * 512:(n + 1) * 512],
                    start=(kt_i == 0), stop=(kt_i == KO - 1),
                    skip_group_check=True)
            ysb = sb_y.tile([128, W], f32, tag="ysb")
            nc.vector.tensor_copy(ysb[:], yp[:])
            nc.sync.dma_start(
                y_d[128 * tt:128 * tt + 128, n * 512:(n + 1) * 512], ysb[:])

        def b_scores(c, p, i, S):
            base = c * W128
            off = 128 * (i - base) if i >= base else 0
            sA = ps_b0.tile([128, W], f32, tag="b0")
            sB = ps_b64.tile([128, W], f32, tag="b64")
            for j, sps in ((0, sA), (1, sB)):
                nc.tensor.matmul(
                    sps[:, off:W],
                    kt[p][64 * j:64 * j + 64, 128 * i:128 * i + 128],
                    qt[p][64 * j:64 * j + 64, c * W + off:(c + 1) * W],
                    start=True, stop=True)
            es = []
            for sps in (sA, sB):
                e = sb_e.tile([128, W], fm, tag="e")
                nc.scalar.activation(e[:, off:W], sps[:, off:W], AF.Exp,
                                     scale=SCALE)
                if i >= base:
                    nc.vector.tensor_mul(e[:, off:off + 128],
                                         e[:, off:off + 128], msk[:])
                es.append(e)
            return es, off

        def b_pv(c, p, i, UA, UB, S, es, off):
            base = c * W128
            last_i = min(S - 1, base + W128 - 1)
            for j, (e, U) in ((0, (es[0], UA)), (1, (es[1], UB))):
                nc.tensor.matmul(
                    U[0:65, off:W],
                    vg[:, i, 2 * p + j, 0:65],
                    e[:, off:W],
                    start=(i == 0), stop=(i == last_i),
                    skip_group_check=True)

        # ---- emission schedule ----
        fillers = deque()

        for p in range(NP):
            qk_job(wq, qt[p], p, 0)
        for p in range(NP):
            qk_job(wk, kt[p], p, 0)
        for ti in range(W128):
            vg_job(ti)

        for c in range(1, NCH):
            for p in range(NP):
                fillers.append(lambda p=p, c=c: qk_job(wq, qt[p], p, c))
            for p in range(NP):
                fillers.append(lambda p=p, c=c: qk_job(wk, kt[p], p, c))
            for ti in range(c * W128, (c + 1) * W128):
                fillers.append(lambda ti=ti: vg_job(ti))

        for c in range(NCH):
            S = (c + 1) * W128
            for p in range(NP):
                UA = ps_u.tile([65, W], f32, tag="UA", name="UA")
                UB = ps_u.tile([65, W], f32, tag="UB", name="UB")
                for i in range(S):
                    es, off = b_scores(c, p, i, S)
                    b_pv(c, p, i, UA, UB, S, es, off)
                    if fillers:
                        fillers.popleft()()
                normalize(c, p, UA, UB)
            for tt in range(c * W128, (c + 1) * W128):
                for n in range(D // 512):
                    fillers.append(lambda tt=tt, n=n: c_job(tt, n))
        while fillers:
            fillers.popleft()()

    nc.compile()
    meta = dict(T=T, D=D, h_loc=h_loc, dh=dh, W=W)
    return nc, meta


def prepare_core_inputs(x, W_qkv, b_qkv, W_g, W_out, b_out,
                        T=T_FULL, D=D_MODEL, h_loc=H_LOC, dh=D_HEAD):
    """Host-side sharding: returns list of per-core input dicts."""
    x = np.asarray(x, dtype=np.float32)
    W_qkv = np.asarray(W_qkv, dtype=np.float32)
    W_g = np.asarray(W_g, dtype=np.float32)
    W_out = np.asarray(W_out, dtype=np.float32)
    KN = D // 128
    DHL = h_loc * dh
    KO = DHL // 128
    n_groups = N_CORES // B
    mask = np.ascontiguousarray(
        (np.arange(128)[:, None] <= np.arange(128)[None, :]).astype(np.float32))

    in_maps = []
    for core in range(N_CORES):
        b, g = divmod(core, n_groups)
        cols = slice(DHL * g, DHL * (g + 1))
        xt = np.ascontiguousarray(
            x[b].T.reshape(KN, 128, T).transpose(1, 0, 2))
        wq = np.ascontiguousarray(
            W_qkv[:, 0 * D:1 * D][:, cols].reshape(KN, 128, DHL).transpose(1, 0, 2))
        wk = np.ascontiguousarray(
            W_qkv[:, 1 * D:2 * D][:, cols].reshape(KN, 128, DHL).transpose(1, 0, 2))
        wv = np.ascontiguousarray(
            W_qkv[:, 2 * D:3 * D][:, cols].reshape(KN, 128, DHL).transpose(1, 0, 2))
        wgh = np.zeros((128, DHL), dtype=np.float32)
        for lh in range(h_loc):
            j = lh % 2
            wgh[64 * j:64 * j + 64, dh * lh:dh * lh + dh] = W_g[h_loc * g + lh]
        wo = np.ascontiguousarray(
            W_out[DHL * g:DHL * (g + 1), :].reshape(KO, 128, D).transpose(1, 0, 2))
        in_maps.append({
            "xt": xt, "wq": wq, "wk": wk, "wv": wv,
            "wg": wgh, "wo": wo, "mask": mask,
            "ones": np.ones((128, T // 128), dtype=np.float32),
        })
    return in_maps


def gather_output(results, b_out):
    """Sum the per-core partial projections into the full output."""
    n_groups = N_CORES // B
    b_out = np.asarray(b_out, dtype=np.float32)
    outs = []
    for b in range(B):
        acc = None
        for g in range(n_groups):
            part = results[b * n_groups + g]["y"]
            acc = part.copy() if acc is None else acc + part
        outs.append(acc + b_out[None, :])
    return np.stack(outs, axis=0)


_BUILD_CACHE = {}


def _get_nc():
    key = (T_FULL, D_MODEL, H_LOC, D_HEAD)
    if key not in _BUILD_CACHE:
        _BUILD_CACHE[key] = build_nc()
    return _BUILD_CACHE[key]


def kernel(x, W_qkv, b_qkv, W_g, W_out, b_out):
    _patch_ldw_opt()
    from concourse.bass_utils import run_bass_kernel_spmd

    b_qkv = np.asarray(b_qkv, dtype=np.float32)
    assert not np.any(b_qkv), "nonzero b_qkv not supported by this build"
    nc, _ = _get_nc()
    in_maps = prepare_core_inputs(x, W_qkv, b_qkv, W_g, W_out, b_out)
    res = run_bass_kernel_spmd(nc, in_maps, core_ids=list(range(N_CORES)))
    return gather_output(res.results, b_out).astype(np.float32)



# revision 8
# speedup vs baseline: 1.0486x; 1.0486x over previous
"""Bass/Trainium2 kernel for BilinearlyModulatedAttention (v2, bf16).

Sharding: 8 cores = 2 (batch) x 4 (head groups of 4 heads).
Each core computes, for its batch b and heads [4g, 4g+4):
  QT/KT (feature-major, d x T), V (token-major), bilinear gate, causal
  softmax in transposed layout (scores[s, t]), PV with a ones-column
  giving softmax denominators, normalization, and a partial output
  projection Y_partial = O^T.T @ W_out[rows]. Host sums the 4 partials
  per batch and adds b_out.

v2 changes vs the fp32r baseline (283-297us):
 - ALL matmul operands in bf16 (fp32r measured ~2 cycles/row on HW at
   N=512 warm: 428ns avg; bf16 streams 1 cycle/row and enables FWL
   fast weight loads). PSUM accumulation stays fp32.
 - scores for the two heads of a pair go into ONE 2-bank psum tile
   ([128,2,512]: bank0 = row-group-0 head, bank1 = row-group-64 head,
   satisfying the one-row-group-base-per-bank rule), so exp is ONE
   ACT instruction over both heads (halves ACT instruction count).
 - input DMAs coalesced (~20 instead of ~40) and split across the
   sync + gpsimd queues; x arrives per-k chunk-pair so the first
   matmul starts ~3us in (baseline was DMA-starved for 40us).
 - psum: scores pool [128,2,512]x2 (4 banks) reserved for b_iters,
   filler pool x1 (2 banks) for qk/vg/out-proj jobs, U pool [65,2,512]
   x1 (2 banks). 8 banks total.
 - softmax denominator row copies go through ACT (Copy) instead of DVE;
   normalize DMA bounces ride the gpsimd queue.
 - out-projection does both 512-col slabs per token tile in one job
   (one DVE drain + one y DMA per 128 tokens).
"""

import sys

if "/opt/trn_rl_repo" not in sys.path:
    sys.path.insert(0, "/opt/trn_rl_repo")

import numpy as np

D_MODEL = 1024
N_HEADS = 16
D_HEAD = 64
B = 2
T_FULL = 2048
N_CORES = 8
H_LOC = N_HEADS // (N_CORES // B)  # 4 heads per core

_LDW_PATCHED = False


def _patch_ldw_opt():
    """Compile walrus with --enable-ldw-opt=true (elides redundant
    LDWEIGHTS reloads). Wraps concourse.bass_utils.run_command."""
    global _LDW_PATCHED
    if _LDW_PATCHED:
        return
    import concourse.bass_utils as BU
    orig = BU.run_command

    def run_patched(argv, **kw):
        argv = [a.replace("--enable-ldw-opt=false", "--enable-ldw-opt=true")
                if isinstance(a, str) else a for a in argv]
        return orig(argv, **kw)

    BU.run_command = run_patched
    _LDW_PATCHED = True


def build_nc(T=T_FULL, D=D_MODEL, h_loc=H_LOC, dh=D_HEAD, W=512):
    """Build the Bass module for one core's shard. Returns (nc, meta)."""
    import concourse.bass as bass
    import concourse.mybir as mybir
    import concourse.tile as tile
    from concourse import bacc
    from contextlib import ExitStack
    from collections import deque

    f32 = mybir.dt.float32
    bf = mybir.dt.bfloat16
    AF = mybir.ActivationFunctionType
    ALU = mybir.AluOpType

    KN = D // 128             # k-tiles for the qkv projections
    TT = T // 128             # 128-token tiles
    assert T % W == 0 and W == 512
    NCH = T // W              # chunks
    CPW = min(2, NCH)         # chunks per qk/projection job
    NCP = (NCH + CPW - 1) // CPW  # chunk-pair jobs
    W128 = W // 128           # s-tiles per chunk (4)
    DHL = h_loc * dh          # local head dim total (256)
    NP = h_loc // 2           # head pairs
    KO = DHL // 128           # out-proj k-tiles (2)
    VGW = dh + 1              # vg last dim: 64 V cols + ones col
    SCALE = 1.0 / float(np.sqrt(dh))

    nc = bacc.Bacc("TRN2", target_bir_lowering=False, debug=False)

    xt_d = nc.dram_tensor("xt", (128, KN, T), bf, kind="ExternalInput")
    wq_d = nc.dram_tensor("wq", (128, KN, DHL), bf, kind="ExternalInput")
    wk_d = nc.dram_tensor("wk", (128, KN, DHL), bf, kind="ExternalInput")
    wv_d = nc.dram_tensor("wv", (128, KN, DHL), bf, kind="ExternalInput")
    wg_d = nc.dram_tensor("wg", (128, 2 * dh), bf, kind="ExternalInput")
    wo_d = nc.dram_tensor("wo", (128, KO, D), bf, kind="ExternalInput")
    mask_d = nc.dram_tensor("mask", (128, 2, 128), bf, kind="ExternalInput")
    ones_d = nc.dram_tensor("ones", (128, TT), bf, kind="ExternalInput")
    y_d = nc.dram_tensor("y", (T, D), f32, kind="ExternalOutput")

    with ExitStack() as ctx:
        tc = ctx.enter_context(tile.TileContext(nc))
        sb_w = ctx.enter_context(tc.tile_pool(name="wts", bufs=1))
        sb_big = ctx.enter_context(tc.tile_pool(name="big", bufs=1))
        sb_e = ctx.enter_context(tc.tile_pool(name="e", bufs=4))
        sb_sig = ctx.enter_context(tc.tile_pool(name="sig", bufs=2))
        sb_nrm = ctx.enter_context(tc.tile_pool(name="nrm", bufs=2))
        sb_y = ctx.enter_context(tc.tile_pool(name="ysb", bufs=3))
        ps_sc = ctx.enter_context(
            tc.tile_pool(name="pssc", bufs=2, space=bass.MemorySpace.PSUM))
        ps_f = ctx.enter_context(
            tc.tile_pool(name="psf", bufs=1, space=bass.MemorySpace.PSUM))
        ps_u = ctx.enter_context(
            tc.tile_pool(name="psu", bufs=1, space=bass.MemorySpace.PSUM))

        # ---- persistent SBUF tensors ----
        xt = sb_big.tile([128, KN, T], bf, tag="xt")
        wq = sb_w.tile([128, KN, DHL], bf, tag="wq")
        wk = sb_w.tile([128, KN, DHL], bf, tag="wk")
        wv = sb_w.tile([128, KN, DHL], bf, tag="wv")
        wg = sb_w.tile([128, 2 * dh], bf, tag="wg")
        wo = sb_w.tile([128, KO, D], bf, tag="wo")
        msk = sb_w.tile([128, 2, 128], bf, tag="msk")
        qt = [sb_big.tile([128, T], bf, tag=f"qt{p}", name=f"qt{p}")
              for p in range(NP)]
        kt = [sb_big.tile([128, T], bf, tag=f"kt{p}", name=f"kt{p}")
              for p in range(NP)]
        ot = [sb_big.tile([128, T], bf, tag=f"ot{p}", name=f"ot{p}")
              for p in range(NP)]
        # vg head index is jp = 2*j + p  (j = row-group half, p = pair)
        vg = sb_big.tile([128, TT, h_loc, VGW], bf, tag="vg")

        # ---- input DMAs ----
        # sync queue: what phase A needs first (wq, then xt chunk-pair 0
        # per k-plane so the first qk_job pipelines, then wk).
        nc.sync.dma_start(wq[:], wq_d[:])
        for k in range(KN):
            nc.sync.dma_start(xt[:, k, 0:CPW * W], xt_d[:, k, 0:CPW * W])
        nc.sync.dma_start(wk[:], wk_d[:])
        # gpsimd queue: V/gate weights, masks, the back half of x, wo.
        nc.gpsimd.dma_start(wv[:], wv_d[:])
        nc.gpsimd.dma_start(wg[:], wg_d[:])
        nc.gpsimd.dma_start(msk[:], mask_d[:])
        for jp in range(h_loc):
            nc.gpsimd.dma_start(vg[:, :, jp, dh], ones_d[:])
        if T > CPW * W:
            nc.gpsimd.dma_start(xt[:, :, CPW * W:T], xt_d[:, :, CPW * W:T])
        nc.gpsimd.dma_start(wo[:], wo_d[:])

        # ---- phase-A jobs ----
        def qk_job(w_sb, dst, p, cp, pool):
            # one chunk-pair (CPW*W cols) per job; the chunks share each
            # k's LDWEIGHTS (consecutive same-lhsT matmuls are elided by
            # --enable-ldw-opt=true)
            ps = pool.tile([128, 2, W], f32, tag=pool._qk_tag)
            for k in range(KN):
                for cc in range(CPW):
                    nc.tensor.matmul(
                        ps[:, cc, :], w_sb[:, k, 128 * p:128 * p + 128],
                        xt[:, k, (CPW * cp + cc) * W:(CPW * cp + cc + 1) * W],
                        start=(k == 0), stop=(k == KN - 1),
                        skip_group_check=True)
            nc.vector.tensor_copy(
                dst[:, CPW * cp * W:CPW * (cp + 1) * W]
                .rearrange("p (a b) -> p a b", a=CPW),
                ps[:, 0:CPW, :])

        def vg_job(ti, pool):
            # one 2-bank psum tile: bank0 = V (cols 0:DHL, jp-ordered) +
            # j=0 gates (cols DHL:DHL+128); bank1 = j=1 gates (cols 0:128).
            vps = pool.tile([128, 2, W], f32, tag=pool._qk_tag)
            for k in range(KN):
                nc.tensor.matmul(
                    vps[:, 0, 0:DHL],
                    xt[:, k, 128 * ti:128 * ti + 128],
                    wv[:, k, :],
                    start=(k == 0), stop=(k == KN - 1),
                    skip_group_check=True)
            for p in range(NP):
                nc.tensor.matmul(
                    vps[:, 0, DHL + 64 * p:DHL + 64 * p + 64],
                    qt[p][0:64, 128 * ti:128 * ti + 128],
                    wg[0:64, dh * p:dh * p + dh],
                    start=True, stop=True, skip_group_check=True)
            for p in range(NP):
                nc.tensor.matmul(
                    vps[:, 1, 64 * p:64 * p + 64],
                    qt[p][64:128, 128 * ti:128 * ti + 128],
                    wg[64:128, dh * p:dh * p + dh],
                    start=True, stop=True, skip_group_check=True)
            # sigmoid(x) = 0.5*tanh(x/2) + 0.5 (stays in the exp table set)
            # gate psum offsets: (j0,p0)=DHL, (j0,p1)=DHL+64, (j1,p0)=W,
            # (j1,p1)=W+64 -> uniform [j: stride W-?]... flat view:
            # base DHL, j stride (W + 0) ... offsets: 256,320 | 512,576
            sig = sb_sig.tile([128, 2, 2, dh], bf, tag="sig")
            g_in = (vps[:].rearrange("p a b -> p (a b)")[:, DHL:DHL + 512]
                    .rearrange("p (j x) -> p j x", j=2)[:, :, 0:128])
            nc.scalar.activation(
                sig[:].rearrange("p j pp d -> p j (pp d)"),
                g_in, AF.Tanh, scale=0.5)
            nc.vector.tensor_scalar(sig[:], sig[:], 0.5, 0.5,
                                    ALU.mult, ALU.add)
            nc.vector.tensor_mul(
                vg[:, ti, :, 0:dh],
                vps[:, 0, 0:DHL].rearrange("p (h d) -> p h d", h=h_loc),
                sig[:].rearrange("p j pp d -> p (j pp) d"))

        # ---- phase-B inner iteration ----
        def b_iter(c, p, i, U2, S):
            base = c * W128
            off = 128 * (i - base) if i >= base else 0
            sc = ps_sc.tile([128, 2, W], f32, tag="sc")
            for j in range(2):
                nc.tensor.matmul(
                    sc[:, j, off:W],
                    kt[p][64 * j:64 * j + 64, 128 * i:128 * i + 128],
                    qt[p][64 * j:64 * j + 64, c * W + off:(c + 1) * W],
                    start=True, stop=True)
            e = sb_e.tile([128, 2, W], bf, tag="e")
            nc.scalar.activation(e[:, :, off:W], sc[:, :, off:W], AF.Exp,
                                 scale=SCALE)
            if i >= base:
                nc.vector.tensor_mul(e[:, :, off:off + 128],
                                     e[:, :, off:off + 128], msk[:])
            last_i = min(S - 1, base + W128 - 1)
            for j in range(2):
                nc.tensor.matmul(
                    U2[0:65, j, off:W],
                    vg[:, i, 2 * j + p, 0:65],
                    e[:, j, off:W],
                    start=(i == 0), stop=(i == last_i),
                    skip_group_check=True)

        def normalize(c, p, U2):
            # Denominator rows live at psum partition 64; custom-DVE ops and
            # partition_broadcast need base-0 APs, so bounce via ACT copy +
            # cross-partition SBUF DMA (gpsimd queue, off critical path-ish).
            dt = sb_nrm.tile([65, 2, W], f32, tag="dt")
            nc.scalar.activation(dt[64:65, :, :], U2[64:65, :, :], AF.Copy)
            den2 = sb_nrm.tile([2, W], f32, tag="den")
            nc.gpsimd.dma_start(den2[:], dt[64:65, :, :])
            rec2 = sb_nrm.tile([2, W], f32, tag="rec")
            nc.vector.reciprocal_approx_fast(rec2[:], den2[:])
            recB = sb_nrm.tile([1, W], f32, tag="recB")
            nc.gpsimd.dma_start(recB[:], rec2[1:2, :])
            bcA = sb_nrm.tile([64, W], f32, tag="bcA")
            bcB = sb_nrm.tile([64, W], f32, tag="bcB")
            nc.gpsimd.partition_broadcast(bcA[:], rec2[0:1, :])
            nc.gpsimd.partition_broadcast(bcB[:], recB[:])
            nc.vector.tensor_mul(ot[p][0:64, c * W:(c + 1) * W],
                                 U2[0:64, 0, :], bcA[:])
            obB = sb_nrm.tile([64, W], bf, tag="obB")
            nc.vector.tensor_mul(obB[:], U2[0:64, 1, :], bcB[:])
            nc.gpsimd.dma_start(ot[p][64:128, c * W:(c + 1) * W], obB[:])

        # ---- phase-C job (one 128-token tile, both 512-col slabs) ----
        def c_job(tt, pool, dmaq):
            yp = pool.tile([128, 2, W], f32, tag=pool._qk_tag)
            for kt_i in range(KO):
                for n2 in range(2):
                    nc.tensor.matmul(
                        yp[:, n2, :],
                        ot[kt_i][:, 128 * tt:128 * tt + 128],
                        wo[:, kt_i, n2 * W:(n2 + 1) * W],
                        start=(kt_i == 0), stop=(kt_i == KO - 1),
                        skip_group_check=True)
            ysb = sb_y.tile([128, 2, W], f32, tag="ysb")
            nc.vector.tensor_copy(ysb[:], yp[:])
            dmaq.dma_start(
                y_d[128 * tt:128 * tt + 128, :]
                .rearrange("p (a b) -> p a b", a=2),
                ysb[:])

        ps_sc._qk_tag = "sc"
        ps_f._qk_tag = "f"

        # ---- emission schedule ----
        fillers = deque()

        # upfront: chunk-pair 0 projections + chunk-0 V/gates, alternating
        # between the two psum pools for pipelining.
        up_pools = [ps_sc, ps_f]
        upfront = [
            lambda pl: qk_job(wq, qt[0], 0, 0, pl),
            lambda pl: qk_job(wk, kt[0], 0, 0, pl),
            lambda pl: qk_job(wq, qt[1], 1, 0, pl),
            lambda pl: qk_job(wk, kt[1], 1, 0, pl),
            lambda pl: vg_job(0, pl),
            lambda pl: vg_job(1, pl),
            lambda pl: vg_job(2, pl),
            lambda pl: vg_job(3, pl),
        ]
        for idx, job in enumerate(upfront):
            job(up_pools[idx % 2])

        # fillers: remaining chunk-pair projections interleaved with V/gate
        # tiles (consecutive fillers differ in length to avoid queue stalls).
        af = []
        for cp in range(1, NCP):
            af += [lambda cp=cp: qk_job(wq, qt[0], 0, cp, ps_f),
                   lambda cp=cp: qk_job(wq, qt[1], 1, cp, ps_f),
                   lambda cp=cp: qk_job(wk, kt[0], 0, cp, ps_f),
                   lambda cp=cp: qk_job(wk, kt[1], 1, cp, ps_f)]
        vgf = [lambda ti=ti: vg_job(ti, ps_f) for ti in range(W128, TT)]
        n_mix = min(len(af), len(vgf))
        for i in range(n_mix):
            fillers.append(af[i])
            fillers.append(vgf[i])
        fillers.extend(af[n_mix:])
        fillers.extend(vgf[n_mix:])

        for c in range(NCH):
            S = (c + 1) * W128
            for p in range(NP):
                U2 = ps_u.tile([65, 2, W], f32, tag="U", name="U2")
                for i in range(S):
                    b_iter(c, p, i, U2, S)
                    if fillers:
                        fillers.popleft()()
                normalize(c, p, U2)
            for idx, tt in enumerate(range(c * W128, (c + 1) * W128)):
                if c == NCH - 1:
                    # final chunk's out-proj runs after all b_iters: free to
                    # alternate pools and DMA queues for a pipelined tail.
                    fillers.append(
                        lambda tt=tt, idx=idx: c_job(
                            tt, [ps_f, ps_sc][idx % 2],
                            [nc.sync, nc.gpsimd][idx % 2]))
                else:
                    fillers.append(
                        lambda tt=tt, idx=idx: c_job(
                            tt, ps_f, [nc.sync, nc.gpsimd][idx % 2]))
        while fillers:
            fillers.popleft()()

    nc.compile()
    meta = dict(T=T, D=D, h_loc=h_loc, dh=dh, W=W)
    return nc, meta


def _to_bf16(a):
    import ml_dtypes
    return np.asarray(a, dtype=np.float32).astype(ml_dtypes.bfloat16)


def prepare_core_inputs(x, W_qkv, b_qkv, W_g, W_out, b_out,
                        T=T_FULL, D=D_MODEL, h_loc=H_LOC, dh=D_HEAD):
    """Host-side sharding: returns list of per-core input dicts (bf16)."""
    x = np.asarray(x, dtype=np.float32)
    W_qkv = np.asarray(W_qkv, dtype=np.float32)
    W_g = np.asarray(W_g, dtype=np.float32)
    W_out = np.asarray(W_out, dtype=np.float32)
    KN = D // 128
    DHL = h_loc * dh
    KO = DHL // 128
    NP = h_loc // 2
    n_groups = N_CORES // B
    mask1 = (np.arange(128)[:, None] <= np.arange(128)[None, :]).astype(
        np.float32)
    mask = np.ascontiguousarray(
        np.broadcast_to(mask1[:, None, :], (128, 2, 128)))
    # jp order: jp = 2*j + p  ->  head h = 2*p + j
    jp_heads = [2 * (m % NP) + (m // NP) for m in range(h_loc)]

    in_maps = []
    for core in range(N_CORES):
        b, g = divmod(core, n_groups)
        cols = slice(DHL * g, DHL * (g + 1))
        xt = np.ascontiguousarray(
            x[b].T.reshape(KN, 128, T).transpose(1, 0, 2))
        wq = np.ascontiguousarray(
            W_qkv[:, 0 * D:1 * D][:, cols].reshape(KN, 128, DHL)
            .transpose(1, 0, 2))
        wk = np.ascontiguousarray(
            W_qkv[:, 1 * D:2 * D][:, cols].reshape(KN, 128, DHL)
            .transpose(1, 0, 2))
        wv_cols = W_qkv[:, 2 * D:3 * D][:, cols]
        wv_r = np.concatenate(
            [wv_cols[:, dh * h:dh * h + dh] for h in jp_heads], axis=1)
        wv = np.ascontiguousarray(
            wv_r.reshape(KN, 128, DHL).transpose(1, 0, 2))
        wgh = np.zeros((128, 2 * dh), dtype=np.float32)
        for j in range(2):
            for p in range(NP):
                wgh[64 * j:64 * j + 64, dh * p:dh * p + dh] = \
                    W_g[h_loc * g + 2 * p + j]
        wo = np.ascontiguousarray(
            W_out[DHL * g:DHL * (g + 1), :].reshape(KO, 128, D)
            .transpose(1, 0, 2))
        in_maps.append({
            "xt": _to_bf16(xt), "wq": _to_bf16(wq), "wk": _to_bf16(wk),
            "wv": _to_bf16(wv), "wg": _to_bf16(wgh), "wo": _to_bf16(wo),
            "mask": _to_bf16(mask),
            "ones": _to_bf16(np.ones((128, T // 128), dtype=np.float32)),
        })
    return in_maps


def gather_output(results, b_out):
    """Sum the per-core partial projections into the full output."""
    n_groups = N_CORES // B
    b_out = np.asarray(b_out, dtype=np.float32)
    outs = []
    for b in range(B):
        acc = None
        for g in range(n_groups):
            part = np.asarray(results[b * n_groups + g]["y"],
                              dtype=np.float32)
            acc = part.copy() if acc is None else acc + part
        outs.append(acc + b_out[None, :])
    return np.stack(outs, axis=0)


_BUILD_CACHE = {}


def _get_nc():
    key = (T_FULL, D_MODEL, H_LOC, D_HEAD)
    if key not in _BUILD_CACHE:
        _BUILD_CACHE[key] = build_nc()
    return _BUILD_CACHE[key]


def kernel(x, W_qkv, b_qkv, W_g, W_out, b_out):
    _patch_ldw_opt()
    from concourse.bass_utils import run_bass_kernel_spmd

    b_qkv = np.asarray(b_qkv, dtype=np.float32)
    assert not np.any(b_qkv), "nonzero b_qkv not supported by this build"
    nc, _ = _get_nc()
    in_maps = prepare_core_inputs(x, W_qkv, b_qkv, W_g, W_out, b_out)
    res = run_bass_kernel_spmd(nc, in_maps, core_ids=list(range(N_CORES)))
    return gather_output(res.results, b_out).astype(np.float32)


# revision 15
# speedup vs baseline: 1.3588x; 1.2958x over previous
"""Bass/Trainium2 kernel for BilinearlyModulatedAttention (v2, bf16).

Sharding: 8 cores = 2 (batch) x 4 (head groups of 4 heads).
Each core computes, for its batch b and heads [4g, 4g+4):
  QT/KT (feature-major, d x T), V (token-major), bilinear gate, causal
  softmax in transposed layout (scores[s, t]), PV with a ones-column
  giving softmax denominators, normalization, and a partial output
  projection Y_partial = O^T.T @ W_out[rows]. Host sums the 4 partials
  per batch and adds b_out.

v2 changes vs the fp32r baseline (283-297us):
 - ALL matmul operands in bf16 (fp32r measured ~2 cycles/row on HW at
   N=512 warm: 428ns avg; bf16 streams 1 cycle/row and enables FWL
   fast weight loads). PSUM accumulation stays fp32.
 - scores for the two heads of a pair go into ONE 2-bank psum tile
   ([128,2,512]: bank0 = row-group-0 head, bank1 = row-group-64 head,
   satisfying the one-row-group-base-per-bank rule), so exp is ONE
   ACT instruction over both heads (halves ACT instruction count).
 - input DMAs coalesced (~20 instead of ~40) and split across the
   sync + gpsimd queues; x arrives per-k chunk-pair so the first
   matmul starts ~3us in (baseline was DMA-starved for 40us).
 - psum: scores pool [128,2,512]x2 (4 banks) reserved for b_iters,
   filler pool x1 (2 banks) for qk/vg/out-proj jobs, U pool [65,2,512]
   x1 (2 banks). 8 banks total.
 - softmax denominator row copies go through ACT (Copy) instead of DVE;
   normalize DMA bounces ride the gpsimd queue.
 - out-projection does both 512-col slabs per token tile in one job
   (one DVE drain + one y DMA per 128 tokens).
"""

import sys

if "/opt/trn_rl_repo" not in sys.path:
    sys.path.insert(0, "/opt/trn_rl_repo")

import numpy as np

D_MODEL = 1024
N_HEADS = 16
D_HEAD = 64
B = 2
T_FULL = 2048
N_CORES = 8
H_LOC = N_HEADS // (N_CORES // B)  # 4 heads per core

_LDW_PATCHED = False


def _patch_ldw_opt():
    """Compile walrus with --enable-ldw-opt=true (elides redundant
    LDWEIGHTS reloads). Wraps concourse.bass_utils.run_command."""
    global _LDW_PATCHED
    if _LDW_PATCHED:
        return
    import concourse.bass_utils as BU
    orig = BU.run_command

    def run_patched(argv, **kw):
        argv = [a.replace("--enable-ldw-opt=false", "--enable-ldw-opt=true")
                if isinstance(a, str) else a for a in argv]
        return orig(argv, **kw)

    BU.run_command = run_patched
    _LDW_PATCHED = True


def build_nc(T=T_FULL, D=D_MODEL, h_loc=H_LOC, dh=D_HEAD, W=512):
    """Build the Bass module for one core's shard. Returns (nc, meta)."""
    import concourse.bass as bass
    import concourse.mybir as mybir
    import concourse.tile as tile
    from concourse import bacc
    from contextlib import ExitStack
    from collections import deque

    f32 = mybir.dt.float32
    bf = mybir.dt.bfloat16
    AF = mybir.ActivationFunctionType
    ALU = mybir.AluOpType

    KN = D // 128             # k-tiles for the qkv projections
    TT = T // 128             # 128-token tiles
    assert T % W == 0 and W == 512
    NCH = T // W              # chunks
    CPW = min(2, NCH)         # chunks per qk/projection job
    NCP = (NCH + CPW - 1) // CPW  # chunk-pair jobs
    W128 = W // 128           # s-tiles per chunk (4)
    DHL = h_loc * dh          # local head dim total (256)
    NP = h_loc // 2           # head pairs
    KO = DHL // 128           # out-proj k-tiles (2)
    VGW = dh + 1              # vg last dim: 64 V cols + ones col
    SCALE = 1.0 / float(np.sqrt(dh))

    nc = bacc.Bacc("TRN2", target_bir_lowering=False, debug=False)

    xt_d = nc.dram_tensor("xt", (128, KN, T), bf, kind="ExternalInput")
    wq_d = nc.dram_tensor("wq", (128, KN, DHL), bf, kind="ExternalInput")
    wk_d = nc.dram_tensor("wk", (128, KN, DHL), bf, kind="ExternalInput")
    wv_d = nc.dram_tensor("wv", (128, KN, DHL), bf, kind="ExternalInput")
    wg_d = nc.dram_tensor("wg", (128, 2 * dh), bf, kind="ExternalInput")
    wo_d = nc.dram_tensor("wo", (128, KO, D), bf, kind="ExternalInput")
    mask_d = nc.dram_tensor("mask", (128, 2, 128), bf, kind="ExternalInput")
    ones_d = nc.dram_tensor("ones", (128, TT), bf, kind="ExternalInput")
    y_d = nc.dram_tensor("y", (T, D), f32, kind="ExternalOutput")

    with ExitStack() as ctx:
        tc = ctx.enter_context(tile.TileContext(nc))
        sb_w = ctx.enter_context(tc.tile_pool(name="wts", bufs=1))
        sb_big = ctx.enter_context(tc.tile_pool(name="big", bufs=1))
        sb_e = ctx.enter_context(tc.tile_pool(name="e", bufs=4))
        sb_sig = ctx.enter_context(tc.tile_pool(name="sig", bufs=2))
        sb_nrm = ctx.enter_context(tc.tile_pool(name="nrm", bufs=2))
        sb_y = ctx.enter_context(tc.tile_pool(name="ysb", bufs=3))
        ps_sc = ctx.enter_context(
            tc.tile_pool(name="pssc", bufs=2, space=bass.MemorySpace.PSUM))
        ps_f = ctx.enter_context(
            tc.tile_pool(name="psf", bufs=1, space=bass.MemorySpace.PSUM))
        ps_u = ctx.enter_context(
            tc.tile_pool(name="psu", bufs=1, space=bass.MemorySpace.PSUM))

        # ---- persistent SBUF tensors ----
        xt = sb_big.tile([128, KN, T], bf, tag="xt")
        wq = sb_w.tile([128, KN, DHL], bf, tag="wq")
        wk = sb_w.tile([128, KN, DHL], bf, tag="wk")
        wv = sb_w.tile([128, KN, DHL], bf, tag="wv")
        wg = sb_w.tile([128, 2 * dh], bf, tag="wg")
        wo = sb_w.tile([128, KO, D], bf, tag="wo")
        msk = sb_w.tile([128, 2, 128], bf, tag="msk")
        qt = [sb_big.tile([128, T], bf, tag=f"qt{p}", name=f"qt{p}")
              for p in range(NP)]
        kt = [sb_big.tile([128, T], bf, tag=f"kt{p}", name=f"kt{p}")
              for p in range(NP)]
        ot = [sb_big.tile([128, T], bf, tag=f"ot{p}", name=f"ot{p}")
              for p in range(NP)]
        # vg head index is jp = 2*j + p  (j = row-group half, p = pair)
        vg = sb_big.tile([128, TT, h_loc, VGW], bf, tag="vg")

        # ---- input DMAs ----
        # sync queue: what phase A needs first (wq, then xt chunk-pair 0
        # per k-plane so the first qk_job pipelines, then wk).
        nc.sync.dma_start(wq[:], wq_d[:])
        for k in range(KN):
            nc.sync.dma_start(xt[:, k, 0:CPW * W], xt_d[:, k, 0:CPW * W])
        nc.sync.dma_start(wk[:], wk_d[:])
        # gpsimd queue: V/gate weights, masks, the back half of x, wo.
        nc.gpsimd.dma_start(wv[:], wv_d[:])
        nc.gpsimd.dma_start(wg[:], wg_d[:])
        nc.gpsimd.dma_start(msk[:], mask_d[:])
        for jp in range(h_loc):
            nc.gpsimd.dma_start(vg[:, :, jp, dh], ones_d[:])
        if T > CPW * W:
            nc.gpsimd.dma_start(xt[:, :, CPW * W:T], xt_d[:, :, CPW * W:T])
        nc.gpsimd.dma_start(wo[:], wo_d[:])

        # ---- phase-A jobs ----
        def qk_job(w_sb, dst, p, c0, ncc, pool):
            # ncc consecutive chunks per job; the chunks share each k's
            # LDWEIGHTS (consecutive same-lhsT matmuls are elided by
            # --enable-ldw-opt=true)
            ps = pool.tile([128, 2, W], f32, tag=pool._qk_tag)
            for k in range(KN):
                for cc in range(ncc):
                    nc.tensor.matmul(
                        ps[:, cc, :], w_sb[:, k, 128 * p:128 * p + 128],
                        xt[:, k, (c0 + cc) * W:(c0 + cc + 1) * W],
                        start=(k == 0), stop=(k == KN - 1),
                        skip_group_check=True)
            nc.vector.tensor_copy(
                dst[:, c0 * W:(c0 + ncc) * W]
                .rearrange("p (a b) -> p a b", a=ncc),
                ps[:, 0:ncc, :])

        def vg_job(ti, pool):
            # one 2-bank psum tile: bank0 = V (cols 0:DHL, jp-ordered) +
            # j=0 gates (cols DHL:DHL+128); bank1 = j=1 gates (cols 0:128).
            vps = pool.tile([128, 2, W], f32, tag=pool._qk_tag)
            for k in range(KN):
                nc.tensor.matmul(
                    vps[:, 0, 0:DHL],
                    xt[:, k, 128 * ti:128 * ti + 128],
                    wv[:, k, :],
                    start=(k == 0), stop=(k == KN - 1),
                    skip_group_check=True)
            for p in range(NP):
                nc.tensor.matmul(
                    vps[:, 0, DHL + 64 * p:DHL + 64 * p + 64],
                    qt[p][0:64, 128 * ti:128 * ti + 128],
                    wg[0:64, dh * p:dh * p + dh],
                    start=True, stop=True, skip_group_check=True)
            for p in range(NP):
                nc.tensor.matmul(
                    vps[:, 1, 64 * p:64 * p + 64],
                    qt[p][64:128, 128 * ti:128 * ti + 128],
                    wg[64:128, dh * p:dh * p + dh],
                    start=True, stop=True, skip_group_check=True)
            # sigmoid(x) = 0.5*tanh(x/2) + 0.5 (stays in the exp table set)
            # gate psum offsets: (j0,p0)=DHL, (j0,p1)=DHL+64, (j1,p0)=W,
            # (j1,p1)=W+64 -> uniform [j: stride W-?]... flat view:
            # base DHL, j stride (W + 0) ... offsets: 256,320 | 512,576
            sig = sb_sig.tile([128, 2, 2, dh], bf, tag="sig")
            g_in = (vps[:].rearrange("p a b -> p (a b)")[:, DHL:DHL + 512]
                    .rearrange("p (j x) -> p j x", j=2)[:, :, 0:128])
            nc.scalar.activation(
                sig[:].rearrange("p j pp d -> p j (pp d)"),
                g_in, AF.Tanh, scale=0.5)
            nc.vector.tensor_scalar(sig[:], sig[:], 0.5, 0.5,
                                    ALU.mult, ALU.add)
            nc.vector.tensor_mul(
                vg[:, ti, :, 0:dh],
                vps[:, 0, 0:DHL].rearrange("p (h d) -> p h d", h=h_loc),
                sig[:].rearrange("p j pp d -> p (j pp) d"))

        # ---- phase-B inner iteration ----
        def b_iter(c, p, i, U2, S):
            base = c * W128
            off = 128 * (i - base) if i >= base else 0
            sc = ps_sc.tile([128, 2, W], f32, tag="sc")
            for j in range(2):
                nc.tensor.matmul(
                    sc[:, j, off:W],
                    kt[p][64 * j:64 * j + 64, 128 * i:128 * i + 128],
                    qt[p][64 * j:64 * j + 64, c * W + off:(c + 1) * W],
                    start=True, stop=True)
            e = sb_e.tile([128, 2, W], bf, tag="e")
            if off == 0:
                # contiguous 1D view over both banks: cheaper AP for ACT
                nc.scalar.activation(
                    e[:].rearrange("p a b -> p (a b)"),
                    sc[:].rearrange("p a b -> p (a b)"),
                    AF.Exp, scale=SCALE)
            else:
                nc.scalar.activation(e[:, :, off:W], sc[:, :, off:W],
                                     AF.Exp, scale=SCALE)
            if i >= base:
                nc.vector.tensor_mul(e[:, :, off:off + 128],
                                     e[:, :, off:off + 128], msk[:])
            last_i = min(S - 1, base + W128 - 1)
            for j in range(2):
                nc.tensor.matmul(
                    U2[0:65, j, off:W],
                    vg[:, i, 2 * j + p, 0:65],
                    e[:, j, off:W],
                    start=(i == 0), stop=(i == last_i),
                    skip_group_check=True)

        def normalize(c, p, U2):
            # Drain U raw to SBUF first so the single-buffered U psum frees
            # after one DVE copy instead of after the whole den/bcast chain.
            uns = sb_nrm.tile([65, 2, W], f32, tag="uns")
            nc.vector.tensor_copy(uns[:], U2[:])
            # Denominator rows live at SBUF partition 64; custom-DVE ops and
            # partition_broadcast need base-0 APs, so bounce via a
            # cross-partition SBUF DMA (sync queue, off the critical path).
            den2 = sb_nrm.tile([2, W], f32, tag="den")
            nc.sync.dma_start(den2[0:1, :], uns[64:65, 0, :])
            nc.sync.dma_start(den2[1:2, :], uns[64:65, 1, :])
            rec2 = sb_nrm.tile([2, W], f32, tag="rec")
            nc.vector.reciprocal_approx_fast(rec2[:], den2[:])
            recB = sb_nrm.tile([1, W], f32, tag="recB")
            nc.sync.dma_start(recB[:], rec2[1:2, :])
            bcA = sb_nrm.tile([64, W], f32, tag="bcA")
            bcB = sb_nrm.tile([64, W], f32, tag="bcB")
            nc.gpsimd.partition_broadcast(bcA[:], rec2[0:1, :])
            nc.gpsimd.partition_broadcast(bcB[:], recB[:])
            nc.vector.tensor_mul(ot[p][0:64, c * W:(c + 1) * W],
                                 uns[0:64, 0, :], bcA[:])
            obB = sb_nrm.tile([64, W], bf, tag="obB")
            nc.vector.tensor_mul(obB[:], uns[0:64, 1, :], bcB[:])
            nc.sync.dma_start(ot[p][64:128, c * W:(c + 1) * W], obB[:])

        # ---- phase-C job (one 128-token tile, both 512-col slabs) ----
        def c_job(tt, pool, dmaq):
            yp = pool.tile([128, 2, W], f32, tag=pool._qk_tag)
            for kt_i in range(KO):
                for n2 in range(2):
                    nc.tensor.matmul(
                        yp[:, n2, :],
                        ot[kt_i][:, 128 * tt:128 * tt + 128],
                        wo[:, kt_i, n2 * W:(n2 + 1) * W],
                        start=(kt_i == 0), stop=(kt_i == KO - 1),
                        skip_group_check=True)
            ysb = sb_y.tile([128, 2, W], f32, tag="ysb")
            nc.vector.tensor_copy(ysb[:], yp[:])
            dmaq.dma_start(
                y_d[128 * tt:128 * tt + 128, :]
                .rearrange("p (a b) -> p a b", a=2),
                ysb[:])

        ps_sc._qk_tag = "sc"
        ps_f._qk_tag = "f"

        # ---- emission schedule ----
        fillers = deque()

        # upfront: chunk-pair 0 projections + chunk-0 V/gates, alternating
        # between the two psum pools for pipelining.
        up_pools = [ps_sc, ps_f]
        upfront = [
            lambda pl: qk_job(wq, qt[0], 0, 0, CPW, pl),
            lambda pl: qk_job(wk, kt[0], 0, 0, CPW, pl),
            lambda pl: qk_job(wq, qt[1], 1, 0, CPW, pl),
            lambda pl: qk_job(wk, kt[1], 1, 0, CPW, pl),
            lambda pl: vg_job(0, pl),
            lambda pl: vg_job(1, pl),
            lambda pl: vg_job(2, pl),
            lambda pl: vg_job(3, pl),
        ]
        for idx, job in enumerate(upfront):
            job(up_pools[idx % 2])

        # fillers carry a deadline: the global b_iter index before which the
        # job MUST be emitted (its output is consumed by that iteration).
        # Jobs are popped when their deadline approaches (margin below), and
        # otherwise paced evenly so the late (big) chunks still get PE work
        # to hide exp latency.
        start_of = [0] * (NCH + 1)
        for c in range(NCH):
            start_of[c + 1] = start_of[c] + (c + 1) * W128 * NP
        total_iters = start_of[NCH]
        MARGIN = 3

        fillers = []  # list of (deadline, job)
        for c in range(CPW, NCH):
            for job in (lambda c=c: qk_job(wq, qt[0], 0, c, 1, ps_f),
                        lambda c=c: qk_job(wq, qt[1], 1, c, 1, ps_f),
                        lambda c=c: qk_job(wk, kt[0], 0, c, 1, ps_f),
                        lambda c=c: qk_job(wk, kt[1], 1, c, 1, ps_f)):
                fillers.append([start_of[c], job])
        for ti in range(W128, TT):
            c1 = ti // W128
            fillers.append([start_of[c1] + (ti % W128),
                            lambda ti=ti: vg_job(ti, ps_f)])
        fillers.sort(key=lambda f: f[0])

        n_filler_est = len(fillers) + NCH * W128  # + c_jobs appended later
        pace = n_filler_est / max(1, total_iters)
        budget = 0.0
        giter = 0

        def pop_fillers():
            nonlocal budget
            while fillers and fillers[0][0] <= giter + MARGIN:
                fillers.pop(0)[1]()
                budget -= 1.0
            while budget >= 1.0 and fillers:
                fillers.pop(0)[1]()
                budget -= 1.0

        for c in range(NCH):
            S = (c + 1) * W128
            for p in range(NP):
                U2 = ps_u.tile([65, 2, W], f32, tag="U", name="U2")
                for i in range(S):
                    b_iter(c, p, i, U2, S)
                    giter += 1
                    budget += pace
                    pop_fillers()
                normalize(c, p, U2)
            for idx, tt in enumerate(range(c * W128, (c + 1) * W128)):
                if c == NCH - 1:
                    # final chunk's out-proj runs after all b_iters: free to
                    # alternate pools and DMA queues for a pipelined tail.
                    fillers.append(
                        [10 ** 9, lambda tt=tt, idx=idx: c_job(
                            tt, [ps_f, ps_sc][idx % 2],
                            [nc.sync, nc.gpsimd][idx % 2])])
                else:
                    fillers.append(
                        [10 ** 9, lambda tt=tt, idx=idx: c_job(
                            tt, ps_f, [nc.sync, nc.gpsimd][idx % 2])])
        while fillers:
            fillers.pop(0)[1]()

    nc.compile()
    meta = dict(T=T, D=D, h_loc=h_loc, dh=dh, W=W)
    return nc, meta


def _to_bf16(a):
    import ml_dtypes
    return np.asarray(a, dtype=np.float32).astype(ml_dtypes.bfloat16)


def prepare_core_inputs(x, W_qkv, b_qkv, W_g, W_out, b_out,
                        T=T_FULL, D=D_MODEL, h_loc=H_LOC, dh=D_HEAD):
    """Host-side sharding: returns list of per-core input dicts (bf16)."""
    x = np.asarray(x, dtype=np.float32)
    W_qkv = np.asarray(W_qkv, dtype=np.float32)
    W_g = np.asarray(W_g, dtype=np.float32)
    W_out = np.asarray(W_out, dtype=np.float32)
    KN = D // 128
    DHL = h_loc * dh
    KO = DHL // 128
    NP = h_loc // 2
    n_groups = N_CORES // B
    mask1 = (np.arange(128)[:, None] <= np.arange(128)[None, :]).astype(
        np.float32)
    mask = np.ascontiguousarray(
        np.broadcast_to(mask1[:, None, :], (128, 2, 128)))
    # jp order: jp = 2*j + p  ->  head h = 2*p + j
    jp_heads = [2 * (m % NP) + (m // NP) for m in range(h_loc)]

    in_maps = []
    for core in range(N_CORES):
        b, g = divmod(core, n_groups)
        cols = slice(DHL * g, DHL * (g + 1))
        xt = np.ascontiguousarray(
            x[b].T.reshape(KN, 128, T).transpose(1, 0, 2))
        wq = np.ascontiguousarray(
            W_qkv[:, 0 * D:1 * D][:, cols].reshape(KN, 128, DHL)
            .transpose(1, 0, 2))
        wk = np.ascontiguousarray(
            W_qkv[:, 1 * D:2 * D][:, cols].reshape(KN, 128, DHL)
            .transpose(1, 0, 2))
        wv_cols = W_qkv[:, 2 * D:3 * D][:, cols]
        wv_r = np.concatenate(
            [wv_cols[:, dh * h:dh * h + dh] for h in jp_heads], axis=1)
        wv = np.ascontiguousarray(
            wv_r.reshape(KN, 128, DHL).transpose(1, 0, 2))
        wgh = np.zeros((128, 2 * dh), dtype=np.float32)
        for j in range(2):
            for p in range(NP):
                wgh[64 * j:64 * j + 64, dh * p:dh * p + dh] = \
                    W_g[h_loc * g + 2 * p + j]
        wo = np.ascontiguousarray(
            W_out[DHL * g:DHL * (g + 1), :].reshape(KO, 128, D)
            .transpose(1, 0, 2))
        in_maps.append({
            "xt": _to_bf16(xt), "wq": _to_bf16(wq), "wk": _to_bf16(wk),
            "wv": _to_bf16(wv), "wg": _to_bf16(wgh), "wo": _to_bf16(wo),
            "mask": _to_bf16(mask),
            "ones": _to_bf16(np.ones((128, T // 128), dtype=np.float32)),
        })
    return in_maps


def gather_output(results, b_out):
    """Sum the per-core partial projections into the full output."""
    n_groups = N_CORES // B
    b_out = np.asarray(b_out, dtype=np.float32)
    outs = []
    for b in range(B):
        acc = None
        for g in range(n_groups):
            part = np.asarray(results[b * n_groups + g]["y"],
                              dtype=np.float32)
            acc = part.copy() if acc is None else acc + part
        outs.append(acc + b_out[None, :])
    return np.stack(outs, axis=0)


_BUILD_CACHE = {}


def _get_nc():
    key = (T_FULL, D_MODEL, H_LOC, D_HEAD)
    if key not in _BUILD_CACHE:
        _BUILD_CACHE[key] = build_nc()
    return _BUILD_CACHE[key]


def kernel(x, W_qkv, b_qkv, W_g, W_out, b_out):
    # NOTE: do NOT enable --enable-ldw-opt with bf16 weights: walrus
    # codegen crashes in visitInstLdweights (FWL + elision conflict).
    from concourse.bass_utils import run_bass_kernel_spmd

    b_qkv = np.asarray(b_qkv, dtype=np.float32)
    assert not np.any(b_qkv), "nonzero b_qkv not supported by this build"
    nc, _ = _get_nc()
    in_maps = prepare_core_inputs(x, W_qkv, b_qkv, W_g, W_out, b_out)
    res = run_bass_kernel_spmd(nc, in_maps, core_ids=list(range(N_CORES)))
    return gather_output(res.results, b_out).astype(np.float32)
